# revision 1
# baseline (speedup 1.0000x reference)
"""GATv2 + Bessel edge-softmax kernel for TRN2, 8-core SPMD, dst-sharded.

Strategy:
  - Sort edges by dst (host). Core k owns node range [k*NV, (k+1)*NV).
  - Per 128-node window: edges padded to fixed block counts, split into
    lo/hi groups by src (int16 gather index limit 32767).
  - On-device: PE projections -> feat tables in DRAM; dma_gather for
    el/er rows; DVE edge pipeline; segment softmax folded into a
    denominator column; scatter-add via one-hot matmul into PSUM.
  - No cross-core communication (output is dst-sharded, concatenated on host).
"""
import sys
sys.path.insert(0, "/opt/trn_rl_repo")
import numpy as np
import ml_dtypes
import concourse.bass as bass
import concourse.tile as tile
from concourse import bacc, mybir
from concourse.bass import ts
from contextlib import ExitStack

F32 = mybir.dt.float32
BF = mybir.dt.bfloat16
I16 = mybir.dt.int16

CUTOFF = 4.0
P_ENV = 7
ENV_A = -(P_ENV + 1) * (P_ENV + 2) / 2.0   # -36
ENV_B = float(P_ENV * (P_ENV + 2))          # 63
ENV_C = -P_ENV * (P_ENV + 1) / 2.0          # -28
H, F, HF, IN = 4, 32, 128, 128


class Cfg:
    def __init__(self, N, n_cores, NV, B_lo, B_hi, V_lo):
        self.N = N                  # real node count
        self.n_cores = n_cores
        self.NV = NV                # padded nodes per core (mult of 128)
        self.NW = NV // 128         # windows per core
        self.N_pad = NV * n_cores
        self.B_lo = B_lo            # lo-gather blocks per window
        self.B_hi = B_hi
        self.B = B_lo + B_hi
        self.V_lo = V_lo            # src table split (<= 32768, mult of 128)
        self.V_hi = self.N_pad - V_lo
        assert self.V_lo % 128 == 0 and self.V_lo <= 32768
        assert self.V_hi <= 32768
        self.S_lo = B_lo * 128 // 16
        self.S_hi = B_hi * 128 // 16
        self.S_d = self.S_lo + self.S_hi


GMAX_BLK = 8   # dma_gather SWDGE ring limit: <=1024 idxs (8 blocks) per call


def wrap_idx(vals, nslots):
    """int16 gather index layout, chunked at GMAX_BLK*128 idxs per call:
    within a chunk, logical i at [i%16, 64*chunk + i//16], replicated across
    the 8 gpsimd groups (rows 16g+r)."""
    a = np.zeros(nslots, np.int32)
    a[: len(vals)] = vals
    assert nslots % 128 == 0
    cols = []
    for c0 in range(0, nslots, GMAX_BLK * 128):
        chunk = a[c0 : c0 + GMAX_BLK * 128]
        w = chunk.reshape(len(chunk) // 16, 16).T.astype(np.int16)  # [16, s]
        cols.append(np.tile(w, (8, 1)))
    return np.concatenate(cols, axis=1)                             # [128, S]


def host_prep(x, distance, W_src, b_src, W_dst, b_dst, attn, prelu_alpha,
              frequencies, src, dst, cfg: Cfg):
    """Build per-core input maps."""
    c = cfg
    E = src.shape[0]
    order = np.argsort(dst, kind="stable")
    src_s, dst_s, dis_s = src[order], dst[order], distance[order]

    BFNP = ml_dtypes.bfloat16
    xT = np.zeros((IN, c.N_pad), BFNP)
    xT[:, : c.N] = x.T.astype(BFNP)

    smalls = dict(
        w_src_t=np.ascontiguousarray(W_src.T.astype(BFNP)),
        w_dst_t=np.ascontiguousarray(W_dst.T.astype(BFNP)),
        b_src_row=b_src.reshape(1, HF).astype(BFNP),
        b_dst_row=b_dst.reshape(1, HF).astype(BFNP),
        attn_row=attn.reshape(1, HF).astype(np.float32),
        alpha_row=prelu_alpha.reshape(1, H).astype(np.float32),
        freq_row=frequencies.reshape(1, H).astype(np.float32),
    )

    # bucket edges by (core, window)
    core_of = dst_s // c.NV
    maps = []
    for k in range(c.n_cores):
        sel = core_of == k
        sk, dk, ik = src_s[sel], dst_s[sel] - k * c.NV, dis_s[sel]
        win = dk // 128
        idx_lo = np.zeros((c.NW, 128, c.S_lo), np.int16)
        idx_hi = np.zeros((c.NW, 128, c.S_hi), np.int16)
        idx_d = np.zeros((c.NW, 128, c.S_d), np.int16)
        # dstw/dist laid out [128, NW, B] so one contiguous DMA covers all
        dstw = np.full((128, c.NW, c.B), -1.0, ml_dtypes.bfloat16)
        dist = np.full((128, c.NW, c.B), 1.0, np.float32)

        for w in range(c.NW):
            wsel = win == w
            ws, wd, wi = sk[wsel], dk[wsel], ik[wsel]
            lo = ws < c.V_lo
            ls, ld, li = ws[lo], wd[lo], wi[lo]
            hs, hd, hi_ = ws[~lo], wd[~lo], wi[~lo]
            n_lo, n_hi = len(ls), len(hs)
            assert n_lo <= c.B_lo * 128, (k, w, n_lo)
            assert n_hi <= c.B_hi * 128, (k, w, n_hi)
            idx_lo[w] = wrap_idx(ls, c.B_lo * 128)
            idx_hi[w] = wrap_idx(hs - c.V_lo, c.B_hi * 128)
            idx_d[w, :, : c.S_lo] = wrap_idx(ld, c.B_lo * 128)
            idx_d[w, :, c.S_lo :] = wrap_idx(hd, c.B_hi * 128)
            # slot j -> (p=j%128, col=j//128); lo slots then hi slots
            jl = np.arange(n_lo)
            dstw[jl % 128, w, jl // 128] = ld - 128 * w
            dist[jl % 128, w, jl // 128] = li
            jh = np.arange(n_hi)
            dstw[jh % 128, w, c.B_lo + jh // 128] = hd - 128 * w
            dist[jh % 128, w, c.B_lo + jh // 128] = hi_

        m = dict(smalls)
        m.update(
            xT=xT,
            xT_own=np.ascontiguousarray(xT[:, k * c.NV : (k + 1) * c.NV]),
            idx_lo=idx_lo, idx_hi=idx_hi, idx_d=idx_d,
            dstw=dstw, dist=dist,
        )
        maps.append(m)
    return maps


def pick_cfg(src, dst, N, n_cores=8):
    NV = ((N + n_cores * 128 - 1) // (n_cores * 128)) * 128
    N_pad = NV * n_cores
    V_lo = min(32768, N_pad // 2 // 128 * 128)
    while N_pad - V_lo > 32768:
        raise ValueError("graph too large for 2-table split")
    order = np.argsort(dst, kind="stable")
    src_s, dst_s = src[order], dst[order]
    core_of = dst_s // NV
    max_lo = max_hi = 1
    for k in range(n_cores):
        sel = core_of == k
        sk, dk = src_s[sel], dst_s[sel] - k * NV
        win = dk // 128
        for w in range(NV // 128):
            ws = sk[win == w]
            n_lo = int((ws < V_lo).sum())
            n_hi = len(ws) - n_lo
            max_lo = max(max_lo, n_lo)
            max_hi = max(max_hi, n_hi)
    B_lo = (max_lo + 127) // 128
    B_hi = (max_hi + 127) // 128
    return Cfg(N, n_cores, NV, B_lo, B_hi, V_lo)


def bcast(t, dims):
    """Manual broadcast AP from a tile/AP: dims is a list of [step, count]."""
    return bass.AP(tensor=t.tensor, offset=t.offset, ap=[list(d) for d in dims])


def build_kernel(c: Cfg):
    nc = bacc.Bacc("TRN2", target_bir_lowering=False, debug=False)
    dp = nc.declare_dram_parameter
    xT = dp("xT", [IN, c.N_pad], BF, isOutput=False)
    xT_own = dp("xT_own", [IN, c.NV], BF, isOutput=False)
    w_src_t = dp("w_src_t", [IN, HF], BF, isOutput=False)
    w_dst_t = dp("w_dst_t", [IN, HF], BF, isOutput=False)
    b_src_row = dp("b_src_row", [1, HF], BF, isOutput=False)
    b_dst_row = dp("b_dst_row", [1, HF], BF, isOutput=False)
    attn_row = dp("attn_row", [1, HF], F32, isOutput=False)
    alpha_row = dp("alpha_row", [1, H], F32, isOutput=False)
    freq_row = dp("freq_row", [1, H], F32, isOutput=False)
    idx_lo = dp("idx_lo", [c.NW, 128, c.S_lo], I16, isOutput=False)
    idx_hi = dp("idx_hi", [c.NW, 128, c.S_hi], I16, isOutput=False)
    idx_d = dp("idx_d", [c.NW, 128, c.S_d], I16, isOutput=False)
    dstw = dp("dstw", [128, c.NW, c.B], BF, isOutput=False)
    dist = dp("dist", [128, c.NW, c.B], F32, isOutput=False)
    out = dp("out", [c.NV, HF], F32, isOutput=True)

    feat_lo = nc.dram_tensor("feat_lo", [c.V_lo, HF], BF)
    feat_hi = nc.dram_tensor("feat_hi", [c.V_hi, HF], BF)
    feat_dst = nc.dram_tensor("feat_dst", [c.NV, HF], BF)

    NWB = c.NW * c.B
    mm = mybir.AluOpType

    with tile.TileContext(nc, pool_alloc_mode="queue") as tc, ExitStack() as ctx:
        con = ctx.enter_context(tc.tile_pool(name="con", bufs=1))
        # --- constants ---
        attn_rep = con.tile([128, HF], F32)
        nc.sync.dma_start(out=attn_rep[:], in_=bcast(attn_row.ap(), [[0, 128], [1, HF]]))
        alpha_rep = con.tile([128, H], F32)
        nc.sync.dma_start(out=alpha_rep[:], in_=bcast(alpha_row.ap(), [[0, 128], [1, H]]))
        # physically replicated per-block copies: contiguous in1 operands keep
        # the DVE 2x (2-port) mode which broadcast (stride-0) APs forfeit
        Bmx = max(c.B_lo, c.B_hi)
        attn_big = con.tile([128, Bmx, HF], BF)
        nc.vector.tensor_copy(
            out=attn_big[:],
            in_=bcast(attn_rep, [list(attn_rep.ap[0]), [0, Bmx], [1, HF]]))
        alpha_big = con.tile([128, Bmx, H, F], BF)
        nc.vector.tensor_copy(
            out=alpha_big[:],
            in_=bcast(alpha_rep, [list(alpha_rep.ap[0]), [0, Bmx], [1, H], [0, F]]))
        freq_rep = con.tile([128, H], F32)
        nc.sync.dma_start(out=freq_rep[:], in_=bcast(freq_row.ap(), [[0, 128], [1, H]]))
        w_src_sb = con.tile([IN, HF], BF)
        nc.sync.dma_start(out=w_src_sb[:], in_=w_src_t[:])
        w_dst_sb = con.tile([IN, HF], BF)
        nc.sync.dma_start(out=w_dst_sb[:], in_=w_dst_t[:])
        ones_sb = con.tile([1, 128], BF)
        nc.vector.memset(ones_sb[:], 1.0)
        b_src_sb = con.tile([1, HF], BF)
        nc.sync.dma_start(out=b_src_sb[:], in_=b_src_row[:])
        b_dst_sb = con.tile([1, HF], BF)
        nc.sync.dma_start(out=b_dst_sb[:], in_=b_dst_row[:])
        iota_i = con.tile([128, 128], mybir.dt.int32)
        nc.gpsimd.iota(iota_i[:], pattern=[[1, 128]], base=0, channel_multiplier=0)
        iota_f = con.tile([128, 128], BF)
        nc.vector.tensor_copy(out=iota_f[:], in_=iota_i[:])
        iota_big = con.tile([128, Bmx, 128], BF)
        nc.vector.tensor_copy(
            out=iota_big[:],
            in_=bcast(iota_f, [list(iota_f.ap[0]), [0, Bmx], [1, 128]]))
        dstw_sb = con.tile([128, c.NW, c.B], BF)
        nc.sync.dma_start(out=dstw_sb[:], in_=dstw[:])
        c_all = con.tile([128, c.NW, c.B, H], BF)

        # --- coefficient pre-pass (Sin table) ---
        with tc.tile_pool(name="coef", bufs=1) as cp:
            d_t = cp.tile([128, NWB], F32)
            nc.sync.dma_start(out=d_t[:], in_=dist.ap().rearrange("p w b -> p (w b)"))
            nc.vector.tensor_scalar_mul(d_t[:], d_t[:], 1.0 / CUTOFF)
            d2 = cp.tile([128, NWB], F32)
            nc.vector.tensor_mul(d2[:], d_t[:], d_t[:])
            d3 = cp.tile([128, NWB], F32)
            nc.vector.tensor_mul(d3[:], d2[:], d_t[:])
            d7 = cp.tile([128, NWB], F32)
            nc.vector.tensor_mul(d7[:], d3[:], d3[:])          # d6
            nc.vector.tensor_mul(d7[:], d7[:], d_t[:])          # d7
            inner = cp.tile([128, NWB], F32)
            nc.vector.tensor_scalar(inner[:], d_t[:], ENV_B, ENV_A, op0=mm.mult, op1=mm.add)
            nc.vector.scalar_tensor_tensor(inner[:], d2[:], ENV_C, inner[:], op0=mm.mult, op1=mm.add)
            env = cp.tile([128, NWB], F32)
            nc.vector.tensor_mul(env[:], d7[:], inner[:])
            nc.vector.tensor_add(env[:], env[:], d_t[:])
            targ = cp.tile([128, NWB, H], F32)
            nc.vector.tensor_tensor(
                out=targ[:], in0=d_t[:].to_broadcast([128, NWB, H]),
                in1=bcast(freq_rep, [list(freq_rep.ap[0]), [0, NWB], [1, H]]),
                op=mm.mult)
            # Sin on ACT only valid in [-pi, pi]; args reach 4*pi -> wrap twice
            nc.vector.add_range_wrap(targ[:], targ[:], 0.0, float(np.pi), float(2 * np.pi))
            nc.vector.add_range_wrap(targ[:], targ[:], 0.0, float(np.pi), float(2 * np.pi))
            sinv = cp.tile([128, NWB, H], F32)
            nc.scalar.activation(sinv[:], targ[:], mybir.ActivationFunctionType.Sin)
            nc.vector.tensor_tensor(
                out=c_all[:], in0=sinv[:],
                in1=env[:].to_broadcast([128, NWB, H]),
                op=mm.mult)

        # --- projections ---
        skip_proj = getattr(c, "skip_proj", False)
        skip_edges = getattr(c, "skip_edges", False)
        abl = getattr(c, "ablate", set())
        with tc.tile_pool(name="proj", bufs=3) as pp, \
             tc.tile_pool(name="projp", bufs=2, space="PSUM") as ppp:
            def project(xt_ap, n_tiles, w_sb, b_sb, dests):
                # dests: list of (dram, tile0, ntiles) covering n_tiles
                G = 16
                for g0 in range(0, n_tiles, G):
                    g = min(G, n_tiles - g0)
                    xt_t = pp.tile([128, G * 128], BF, tag="xt")
                    nc.sync.dma_start(out=xt_t[:, : g * 128],
                                      in_=xt_ap[:, g0 * 128 : (g0 + g) * 128])
                    ps = ppp.tile([128, G * HF], F32)
                    for t in range(g):
                        nc.tensor.matmul(ps[:, ts(t, HF)], lhsT=xt_t[:, ts(t, 128)],
                                         rhs=w_sb[:], start=True, stop=False)
                        nc.tensor.matmul(ps[:, ts(t, HF)], lhsT=ones_sb[:], rhs=b_sb[:],
                                         start=False, stop=True)
                    ft = pp.tile([128, G, HF], BF, tag="ft")
                    nc.scalar.copy(out=ft[:, :g, :].rearrange("p g f -> p (g f)"),
                                   in_=ps[:, : g * HF])
                    for dram, t0, nt in dests:
                        a = max(g0, t0)
                        b = min(g0 + g, t0 + nt)
                        if a < b:
                            rows = dram[(a - t0) * 128 : (b - t0) * 128, :]
                            nc.sync.dma_start(
                                out=rows.rearrange("(t p) f -> p t f", p=128),
                                in_=ft[:, a - g0 : b - g0, :])
            if not skip_proj:
                project(xT.ap(), c.N_pad // 128, w_src_sb, b_src_sb,
                        [(feat_lo, 0, c.V_lo // 128), (feat_hi, c.V_lo // 128, c.V_hi // 128)])
                project(xT_own.ap(), c.NV // 128, w_dst_sb, b_dst_sb,
                        [(feat_dst, 0, c.NV // 128)])

        # --- edge phase ---
        ep = ctx.enter_context(tc.tile_pool(name="edge", bufs=2))
        wp = ctx.enter_context(tc.tile_pool(name="work", bufs=1))
        op_ = ctx.enter_context(tc.tile_pool(name="outp", bufs=2))
        up = ctx.enter_context(tc.tile_pool(name="upsum", bufs=2, space="PSUM"))

        for w in range(c.NW if not skip_edges else 0):
            id_t = ep.tile([128, c.S_d], I16, tag="idxd")
            nc.sync.dma_start(out=id_t[:], in_=idx_d[w])
            U = up.tile([128, HF + H], F32, tag="U")
            for half in range(2):
                P_blk = c.B_lo if half == 0 else c.B_hi
                NP = P_blk * 128
                if half == 0:
                    il_t = ep.tile([128, c.S_lo], I16, tag="idxlo")
                    nc.sync.dma_start(out=il_t[:], in_=idx_lo[w])
                    ftab, ioff, coff = feat_lo, il_t, 0
                else:
                    il_t = ep.tile([128, c.S_hi], I16, tag="idxhi")
                    nc.sync.dma_start(out=il_t[:], in_=idx_hi[w])
                    ftab, ioff, coff = feat_hi, il_t, c.B_lo
                def gchunk(dst, ftab_, it, icol0, nblk):
                    for b0 in range(0, nblk, GMAX_BLK):
                        nb = min(GMAX_BLK, nblk - b0)
                        nc.gpsimd.dma_gather(
                            dst[:, b0 : b0 + nb, :], ftab_[:],
                            it[:, icol0 + b0 * 8 : icol0 + (b0 + nb) * 8],
                            nb * 128, nb * 128, HF)
                el = ep.tile([128, P_blk, HF], BF, tag="el")
                if "gel" not in abl:
                    gchunk(el, ftab, ioff, 0, P_blk)
                else:
                    nc.vector.memset(el[:, :1, :1], 0.0)
                er = ep.tile([128, P_blk, HF], BF, tag="er")
                if "ger" not in abl:
                    gchunk(er, feat_dst, id_t, 0 if half == 0 else c.S_lo, P_blk)
                else:
                    nc.vector.memset(er[:, :1, :1], 0.0)

                s = wp.tile([128, P_blk, H, F], BF, tag="s")
                if "sadd" not in abl:
                    nc.vector.tensor_add(s[:], el[:].rearrange("p b (h f) -> p b h f", h=H),
                                         er[:].rearrange("p b (h f) -> p b h f", h=H))
                else:
                    nc.vector.memset(s[:, :1, :1, :1], 0.0)
                cw = c_all[:, w, coff : coff + P_blk, :]          # [128, P, H]
                c_big = wp.tile([128, P_blk, H, F], BF, tag="cbig")
                nc.scalar.copy(out=c_big[:], in_=cw.to_broadcast([128, P_blk, H, F]))
                if "t1" not in abl:
                    nc.vector.tensor_tensor(out=s[:], in0=s[:], in1=c_big[:], op=mm.mult)
                t2 = wp.tile([128, P_blk, H, F], BF, tag="t2")
                if "t2" not in abl:
                    # prelu: w_h = max(alpha_h * s_h, s_h), alpha per-partition scalar
                    for h in range(H):
                        nc.vector.scalar_tensor_tensor(
                            out=t2[:, :, h, :], in0=s[:, :, h, :],
                            scalar=alpha_rep[:, h : h + 1], in1=s[:, :, h, :],
                            op0=mm.mult, op1=mm.max)
                else:
                    nc.vector.memset(t2[:, :1, :1, :1], 0.0)
                if "attnmul" not in abl:
                    nc.vector.tensor_tensor(
                        out=s[:], in0=t2[:],
                        in1=attn_big[:, :P_blk, :].rearrange("p b (h f) -> p b h f", h=H),
                        op=mm.mult)
                # pairwise bf16 pre-add (2x mode) halves the 1x reduce input
                sh = wp.tile([128, P_blk, H, F // 2], BF, tag="sh")
                nc.vector.tensor_tensor(out=sh[:], in0=s[:, :, :, : F // 2],
                                        in1=s[:, :, :, F // 2 :], op=mm.add)
                score = wp.tile([128, P_blk, H], F32, tag="score")
                if "red" not in abl:
                    nc.vector.tensor_reduce(out=score[:], in_=sh[:], axis=mybir.AxisListType.X, op=mm.add)
                else:
                    nc.vector.memset(score[:, :1, :1], 0.0)
                ex = wp.tile([128, P_blk, H], F32, tag="ex")
                if "exp" not in abl:
                    nc.scalar.activation(ex[:], score[:], mybir.ActivationFunctionType.Exp)
                else:
                    nc.vector.memset(ex[:, :1, :1], 0.0)
                msgex = wp.tile([128, P_blk, HF + H], BF, tag="msgex")
                ex_big = wp.tile([128, P_blk, H, F], BF, tag="exbig")
                nc.scalar.copy(out=ex_big[:], in_=ex[:].to_broadcast([128, P_blk, H, F]))
                if "msgmul" not in abl:
                    nc.vector.tensor_tensor(
                        out=msgex[:, :, :HF].rearrange("p b (h f) -> p b h f", h=H),
                        in0=el[:].rearrange("p b (h f) -> p b h f", h=H),
                        in1=ex_big[:], op=mm.mult)
                else:
                    nc.vector.memset(msgex[:, :1, :1], 0.0)
                if "excopy" not in abl:
                    nc.scalar.copy(out=msgex[:, :, HF:], in_=ex[:])
                oh = wp.tile([128, P_blk, 128], BF, tag="oh")
                dww = dstw_sb[:, w, coff : coff + P_blk]          # [128, P]
                dstw_big = wp.tile([128, P_blk, 128], BF, tag="dwbig")
                nc.scalar.copy(out=dstw_big[:], in_=dww.to_broadcast([128, P_blk, 128]))
                if "oh" not in abl:
                    nc.vector.tensor_tensor(
                        out=oh[:], in0=iota_big[:, :P_blk, :], in1=dstw_big[:],
                        op=mm.is_equal)
                else:
                    nc.vector.memset(oh[:, :1, :1], 0.0)
                if "mm" not in abl:
                    for b in range(P_blk):
                        nc.tensor.matmul(U[:], lhsT=oh[:, b, :], rhs=msgex[:, b, :],
                                         start=(half == 0 and b == 0),
                                         stop=(half == 1 and b == P_blk - 1))
            inv = wp.tile([128, H], F32, tag="inv")
            nc.vector.tensor_scalar_max(inv[:], U[:, HF:], 1e-30)
            nc.vector.reciprocal(inv[:], inv[:])
            ot = op_.tile([128, HF], F32, tag="ot")
            nc.vector.tensor_tensor(
                out=ot[:].rearrange("p (h f) -> p h f", h=H),
                in0=U[:, :HF].rearrange("p (h f) -> p h f", h=H),
                in1=inv[:].to_broadcast([128, H, F]), op=mm.mult)
            nc.sync.dma_start(out=out[ts(w, 128)], in_=ot[:])

    nc.compile()
    return nc


def run_kernel(inputs, cfg=None, trace=False):
    from concourse.bass_utils import run_bass_kernel_spmd
    x = np.asarray(inputs["x"], np.float32)
    src = np.asarray(inputs["src"]); dst = np.asarray(inputs["dst"])
    if cfg is None:
        cfg = pick_cfg(src, dst, x.shape[0], 8)
    maps = host_prep(
        x, np.asarray(inputs["distance"], np.float32),
        np.asarray(inputs["W_src"], np.float32), np.asarray(inputs["b_src"], np.float32),
        np.asarray(inputs["W_dst"], np.float32), np.asarray(inputs["b_dst"], np.float32),
        np.asarray(inputs["attn"], np.float32), np.asarray(inputs["prelu_alpha"], np.float32),
        np.asarray(inputs["frequencies"], np.float32), src, dst, cfg)
    nc = build_kernel(cfg)
    res = run_bass_kernel_spmd(nc, maps, list(range(cfg.n_cores)), trace=trace)
    outs = [res.results[k]["out"] for k in range(cfg.n_cores)]
    full = np.concatenate(outs, axis=0)[: cfg.N]
    return full.reshape(cfg.N, H, F), res


def kernel(**inputs) -> np.ndarray:
    """Full-input entry point: shards across 8 NeuronCores internally."""
    x = np.asarray(inputs["x"], np.float32)
    src = np.asarray(inputs["src"]).astype(np.int64)
    dst = np.asarray(inputs["dst"]).astype(np.int64)
    cfg = pick_cfg(src, dst, x.shape[0], 8)
    maps = host_prep(
        x, np.asarray(inputs["distance"], np.float32),
        np.asarray(inputs["W_src"], np.float32), np.asarray(inputs["b_src"], np.float32),
        np.asarray(inputs["W_dst"], np.float32), np.asarray(inputs["b_dst"], np.float32),
        np.asarray(inputs["attn"], np.float32), np.asarray(inputs["prelu_alpha"], np.float32),
        np.asarray(inputs["frequencies"], np.float32), src, dst, cfg)
    nc = build_kernel(cfg)
    from concourse.bass_utils import run_bass_kernel_spmd
    res = run_bass_kernel_spmd(nc, maps, list(range(cfg.n_cores)))
    outs = [res.results[k]["out"] for k in range(cfg.n_cores)]
    full = np.concatenate(outs, axis=0)[: cfg.N]
    return full.reshape(cfg.N, H, F).astype(np.float32)



# revision 28
# speedup vs baseline: 1.2766x; 1.2766x over previous
"""GATv2 + Bessel edge-softmax kernel for TRN2, 8-core SPMD, dst-sharded.

v2 strategy (vs v1 baseline):
  - Sort edges by dst (host). Core k owns node range [k*NV, (k+1)*NV).
  - Per 128-node window: edges padded to fixed block count B, split lo/hi
    by src only for the el gather (int16 index limit).
  - Feature tables stored PARTITION-MAJOR in DRAM ([128, T, row]) so the
    projection writes stream at 4KB/descriptor; gather indices are
    host-remapped to r' = (s%128)*T + s//128.
  - er gather fused with the scatter one-hot: combined table
    [feat_dst | I] with 512B rows — one gather returns er AND the
    one-hot row used by the scatter matmul (<512B descriptors pay a 2x
    DMA penalty, so the extra 256B is free).
  - One dma_gather call per table per window (994ns SWDGE fixed cost);
    descriptor ring sized 2048 via dynamic_dma_scratch_size=32768.
  - Engine split per window: DVE add/cmul/alphamul/max/preadd/attnmul,
    Pool msgmul(broadcast ex)+reduce+gather prep, ACT c_big bcast + exp,
    PE one-hot scatter matmul into PSUM (numerator + denominator).
  - No cross-core communication (output dst-sharded, concatenated on host).
"""
import sys
sys.path.insert(0, "/opt/trn_rl_repo")
import numpy as np
import ml_dtypes
import concourse.bass as bass
import concourse.tile as tile
from concourse import bacc, mybir
from concourse.bass import ts
from contextlib import ExitStack

F32 = mybir.dt.float32
BF = mybir.dt.bfloat16
I16 = mybir.dt.int16

CUTOFF = 4.0
P_ENV = 7
ENV_A = -(P_ENV + 1) * (P_ENV + 2) / 2.0   # -36
ENV_B = float(P_ENV * (P_ENV + 2))          # 63
ENV_C = -P_ENV * (P_ENV + 1) / 2.0          # -28
H, F, HF, IN = 4, 32, 128, 128


class Cfg:
    def __init__(self, N, n_cores, NV, B_lo, B_hi, V_lo):
        self.N = N                  # real node count
        self.n_cores = n_cores
        self.NV = NV                # padded nodes per core (mult of 128)
        self.NW = NV // 128         # windows per core
        self.N_pad = NV * n_cores
        self.B_lo = B_lo            # lo-gather blocks per window
        self.B_hi = B_hi
        self.B = B_lo + B_hi
        self.V_lo = V_lo            # src table split (<= 32768, mult of 128)
        self.V_hi = self.N_pad - V_lo
        assert self.V_lo % 128 == 0 and self.V_lo <= 32768
        assert self.V_hi <= 32768
        self.T_lo = self.V_lo // 128    # partition-major col count, lo table
        self.T_hi = self.V_hi // 128
        self.T_d = self.NW + 1          # dst table cols (+1 zero pad row)
        self.S_lo = B_lo * 128 // 16    # idx cols per window
        self.S_hi = B_hi * 128 // 16
        self.S_d = self.B * 128 // 16


GMAX = 1024   # HW ucode limit: <=1024 idxs per dma_gather call


def wrap_idx(vals, nslots):
    """int16 gather index layout, chunked at GMAX idxs per call: within a
    chunk, logical i at [i%16, 64*chunk + i//16], replicated across the 8
    gpsimd groups (rows 16g+r)."""
    a = np.zeros(nslots, np.int32)
    a[: len(vals)] = vals
    assert nslots % 128 == 0
    cols = []
    for c0 in range(0, nslots, GMAX):
        chunk = a[c0 : c0 + GMAX]
        w = chunk.reshape(len(chunk) // 16, 16).T.astype(np.int16)  # [16, s]
        cols.append(np.tile(w, (8, 1)))
    return np.concatenate(cols, axis=1)                             # [128, S]


def host_prep(x, distance, W_src, b_src, W_dst, b_dst, attn, prelu_alpha,
              frequencies, src, dst, cfg: Cfg):
    """Build per-core input maps."""
    c = cfg
    order = np.argsort(dst, kind="stable")
    src_s, dst_s, dis_s = src[order], dst[order], distance[order]

    BFNP = ml_dtypes.bfloat16
    xT = np.zeros((IN, c.N_pad), BFNP)
    xT[:, : c.N] = x.T.astype(BFNP)

    # feature columns permuted to (f, h) order: new col f*H+h = old h*F+f.
    # Keeps the packed H dim innermost so per-(edge,head) broadcasts stay in
    # DVE 2x mode.
    perm = (np.arange(HF).reshape(F, H).T.reshape(-1))       # old idx for new? see below
    # old layout hf = h*F+f ; want new[f*H+h] = old[h*F+f]
    perm = np.array([[h * F + f for h in range(H)] for f in range(F)]).reshape(-1)
    smalls = dict(
        w_src_t=np.ascontiguousarray(W_src.T[:, perm].astype(BFNP)),
        w_dst_t=np.ascontiguousarray(W_dst.T[:, perm].astype(BFNP)),
        b_src_row=b_src.reshape(1, HF)[:, perm].astype(BFNP),
        b_dst_row=b_dst.reshape(1, HF)[:, perm].astype(BFNP),
        attn_row=attn.reshape(1, HF)[:, perm].astype(np.float32),
        alpha_row=prelu_alpha.reshape(1, H).astype(np.float32),
        freq_row=frequencies.reshape(1, H).astype(np.float32),
    )

    # partition-major index remap: node s -> row' = (s%128)*T + s//128
    def pm(s, T):
        return (s % 128) * T + s // 128

    core_of = dst_s // c.NV
    maps = []
    for k in range(c.n_cores):
        sel = core_of == k
        sk, dk, ik = src_s[sel], dst_s[sel] - k * c.NV, dis_s[sel]
        win = dk // 128
        idx_lo = np.zeros((c.NW, 128, c.S_lo), np.int16)
        idx_hi = np.zeros((c.NW, 128, c.S_hi), np.int16)
        idx_d = np.zeros((c.NW, 128, c.S_d), np.int16)
        dist = np.full((128, c.NW, c.B), 1.0, np.float32)

        for w in range(c.NW):
            wsel = win == w
            ws, wd, wi = sk[wsel], dk[wsel], ik[wsel]
            lo = ws < c.V_lo
            ls, li = ws[lo], wi[lo]
            hs, hi_ = ws[~lo], wi[~lo]
            ld, hd = wd[lo], wd[~lo]
            n_lo, n_hi = len(ls), len(hs)
            assert n_lo <= c.B_lo * 128, (k, w, n_lo)
            assert n_hi <= c.B_hi * 128, (k, w, n_hi)
            idx_lo[w] = wrap_idx(pm(ls, c.T_lo), c.B_lo * 128)
            idx_hi[w] = wrap_idx(pm(hs - c.V_lo, c.T_hi), c.B_hi * 128)
            # dst rows: local node d -> r' = (d%128)*T_d + d//128; the one-hot
            # part of row r' is onehot(d%128) which is what the scatter needs.
            # padding slots -> column NW (zero row) of partition 0.
            dv = np.full(c.B * 128, c.NW, np.int64)       # pad -> p=0,t=NW
            dv[:n_lo] = pm(ld, c.T_d)
            dv[c.B_lo * 128 : c.B_lo * 128 + n_hi] = pm(hd, c.T_d)
            idx_d[w] = wrap_idx(dv, c.B * 128)
            jl = np.arange(n_lo)
            dist[jl % 128, w, jl // 128] = li
            jh = np.arange(n_hi)
            dist[jh % 128, w, c.B_lo + jh // 128] = hi_

        m = dict(smalls)
        m.update(xT=xT,
                 xT_own=np.ascontiguousarray(xT[:, k * c.NV : (k + 1) * c.NV]),
                 idx_lo=idx_lo, idx_hi=idx_hi, idx_d=idx_d, dist=dist)
        maps.append(m)
    return maps


def pick_cfg(src, dst, N, n_cores=8):
    NV = ((N + n_cores * 128 - 1) // (n_cores * 128)) * 128
    N_pad = NV * n_cores
    V_lo = min(32768, N_pad)
    if N_pad - V_lo > 32768:
        raise ValueError("graph too large for 2-table split")
    order = np.argsort(dst, kind="stable")
    src_s, dst_s = src[order], dst[order]
    core_of = dst_s // NV
    max_lo = max_hi = 1
    for k in range(n_cores):
        sel = core_of == k
        sk, dk = src_s[sel], dst_s[sel] - k * NV
        win = dk // 128
        for w in range(NV // 128):
            ws = sk[win == w]
            n_lo = int((ws < V_lo).sum())
            n_hi = len(ws) - n_lo
            max_lo = max(max_lo, n_lo)
            max_hi = max(max_hi, n_hi)
    B_lo = (max_lo + 127) // 128
    B_hi = (max_hi + 127) // 128
    return Cfg(N, n_cores, NV, B_lo, B_hi, V_lo)


def bcast(t, dims):
    """Manual broadcast AP from a tile/AP: dims is a list of [step, count]."""
    return bass.AP(tensor=t.tensor, offset=t.offset, ap=[list(d) for d in dims])


def build_kernel(c: Cfg):
    nc = bacc.Bacc("TRN2", target_bir_lowering=False, debug=False)
    dp = nc.declare_dram_parameter
    xT = dp("xT", [IN, c.N_pad], BF, isOutput=False)
    xT_own = dp("xT_own", [IN, c.NV], BF, isOutput=False)
    w_src_t = dp("w_src_t", [IN, HF], BF, isOutput=False)
    w_dst_t = dp("w_dst_t", [IN, HF], BF, isOutput=False)
    b_src_row = dp("b_src_row", [1, HF], BF, isOutput=False)
    b_dst_row = dp("b_dst_row", [1, HF], BF, isOutput=False)
    attn_row = dp("attn_row", [1, HF], F32, isOutput=False)
    alpha_row = dp("alpha_row", [1, H], F32, isOutput=False)
    freq_row = dp("freq_row", [1, H], F32, isOutput=False)
    idx_lo = dp("idx_lo", [c.NW, 128, c.S_lo], I16, isOutput=False)
    idx_hi = dp("idx_hi", [c.NW, 128, c.S_hi], I16, isOutput=False)
    idx_d = dp("idx_d", [c.NW, 128, c.S_d], I16, isOutput=False)
    dist = dp("dist", [128, c.NW, c.B], F32, isOutput=False)
    out = dp("out", [c.NV, HF], F32, isOutput=True)

    # partition-major feature tables
    feat_lo = nc.dram_tensor("feat_lo", [128, c.T_lo, HF], BF)
    feat_hi = nc.dram_tensor("feat_hi", [128, c.T_hi, HF], BF)
    # combined dst table: [er row | one-hot row]; col NW is the zero pad row
    feat_doh = nc.dram_tensor("feat_doh", [128, c.T_d, 2 * HF], BF)

    NWB = c.NW * c.B
    mm = mybir.AluOpType

    with tile.TileContext(nc, pool_alloc_mode="queue") as tc, ExitStack() as ctx:
        con = ctx.enter_context(tc.tile_pool(name="con", bufs=1))
        # --- constants ---
        attn_rep = con.tile([128, HF], F32)
        nc.sync.dma_start(out=attn_rep[:], in_=bcast(attn_row.ap(), [[0, 128], [1, HF]]))
        alpha_rep = con.tile([128, H], F32)
        nc.sync.dma_start(out=alpha_rep[:], in_=bcast(alpha_row.ap(), [[0, 128], [1, H]]))
        freq_rep = con.tile([128, H], F32)
        nc.sync.dma_start(out=freq_rep[:], in_=bcast(freq_row.ap(), [[0, 128], [1, H]]))
        # bf16 copies; broadcast over the block dim via stride-0 APs (the
        # packed innermost dim keeps DVE 2x)
        attn_sm = con.tile([128, HF], BF)       # (f,h) order
        nc.vector.tensor_copy(out=attn_sm[:], in_=attn_rep[:])
        alpha_sm = con.tile([128, H], BF)
        nc.vector.tensor_copy(out=alpha_sm[:], in_=alpha_rep[:])
        w_src_sb = con.tile([IN, HF], BF)
        nc.sync.dma_start(out=w_src_sb[:], in_=w_src_t[:])
        w_dst_sb = con.tile([IN, HF], BF)
        nc.sync.dma_start(out=w_dst_sb[:], in_=w_dst_t[:])
        ones_sb = con.tile([1, 128], BF)
        nc.vector.memset(ones_sb[:], 1.0)
        b_src_sb = con.tile([1, HF], BF)
        nc.sync.dma_start(out=b_src_sb[:], in_=b_src_row[:])
        b_dst_sb = con.tile([1, HF], BF)
        nc.sync.dma_start(out=b_dst_sb[:], in_=b_dst_row[:])
        # one-hot identity block for the dst table: I[p, j] = (p == j)
        iota_i = con.tile([128, 128], mybir.dt.int32)
        nc.gpsimd.iota(iota_i[:], pattern=[[1, 128]], base=0, channel_multiplier=0)
        iota_f = con.tile([128, 128], BF)
        nc.vector.tensor_copy(out=iota_f[:], in_=iota_i[:])
        pid_i = con.tile([128, 1], mybir.dt.int32)
        nc.gpsimd.iota(pid_i[:], pattern=[[0, 1]], base=0, channel_multiplier=1)
        pid_f = con.tile([128, 1], BF)
        nc.vector.tensor_copy(out=pid_f[:], in_=pid_i[:])
        eye_sb = con.tile([128, 128], BF)
        nc.vector.tensor_tensor(
            out=eye_sb[:], in0=iota_f[:],
            in1=bcast(pid_f, [list(pid_f.ap[0]), [0, 128]]), op=mm.is_equal)
        zero_row = con.tile([128, 2 * HF], BF)
        nc.vector.memset(zero_row[:], 0.0)
        c_all = con.tile([128, c.NW, c.B, H], BF)

        # write the one-hot halves + zero pad column of feat_doh once
        # (materialized copy: stride-0-source DMA descriptors are not safe)
        with tc.tile_pool(name="eyep", bufs=1) as eyp:
            eye_rep = eyp.tile([128, c.NW, 128], BF)
            nc.vector.tensor_copy(
                out=eye_rep[:],
                in_=bcast(eye_sb, [list(eye_sb.ap[0]), [0, c.NW], [1, 128]]))
            nc.sync.dma_start(out=feat_doh[:, : c.NW, HF:], in_=eye_rep[:])
            nc.sync.dma_start(out=feat_doh[:, c.NW, :], in_=zero_row[:])

        # --- coefficient pre-pass (Sin table) ---
        with tc.tile_pool(name="coef", bufs=1) as cp:
            d_t = cp.tile([128, NWB], F32)
            nc.sync.dma_start(out=d_t[:], in_=dist.ap().rearrange("p w b -> p (w b)"))
            nc.vector.tensor_scalar_mul(d_t[:], d_t[:], 1.0 / CUTOFF)
            d2 = cp.tile([128, NWB], F32)
            nc.vector.tensor_mul(d2[:], d_t[:], d_t[:])
            d3 = cp.tile([128, NWB], F32)
            nc.vector.tensor_mul(d3[:], d2[:], d_t[:])
            d7 = cp.tile([128, NWB], F32)
            nc.vector.tensor_mul(d7[:], d3[:], d3[:])          # d6
            nc.vector.tensor_mul(d7[:], d7[:], d_t[:])          # d7
            inner = cp.tile([128, NWB], F32)
            nc.vector.tensor_scalar(inner[:], d_t[:], ENV_B, ENV_A, op0=mm.mult, op1=mm.add)
            nc.vector.scalar_tensor_tensor(inner[:], d2[:], ENV_C, inner[:], op0=mm.mult, op1=mm.add)
            env = cp.tile([128, NWB], F32)
            nc.vector.tensor_mul(env[:], d7[:], inner[:])
            nc.vector.tensor_add(env[:], env[:], d_t[:])
            targ = cp.tile([128, NWB, H], F32)
            nc.vector.tensor_tensor(
                out=targ[:], in0=d_t[:].to_broadcast([128, NWB, H]),
                in1=bcast(freq_rep, [list(freq_rep.ap[0]), [0, NWB], [1, H]]),
                op=mm.mult)
            # Sin on ACT only valid in [-pi, pi]; args reach 4*pi -> wrap twice
            nc.vector.add_range_wrap(targ[:], targ[:], 0.0, float(np.pi), float(2 * np.pi))
            nc.vector.add_range_wrap(targ[:], targ[:], 0.0, float(np.pi), float(2 * np.pi))
            sinv = cp.tile([128, NWB, H], F32)
            nc.scalar.activation(sinv[:], targ[:], mybir.ActivationFunctionType.Sin)
            nc.vector.tensor_tensor(
                out=c_all[:].rearrange("p w b h -> p (w b) h"), in0=sinv[:],
                in1=env[:].to_broadcast([128, NWB, H]),
                op=mm.mult)

        # --- projections (partition-major tables) ---
        with tc.tile_pool(name="proj", bufs=3) as pp, \
             tc.tile_pool(name="projp", bufs=2, space="PSUM") as ppp:
            def project(src_ap, n_tiles, w_sb, b_sb, dests):
                # dests: list of (dram_slice_fn, t0, nt); dram_slice_fn(a, b)
                # yields the out AP for table cols [a-t0, b-t0)
                G = 16
                for g0 in range(0, n_tiles, G):
                    g = min(G, n_tiles - g0)
                    xt_t = pp.tile([128, G * 128], BF, tag="xt")
                    nc.sync.dma_start(
                        out=xt_t[:, : g * 128],
                        in_=src_ap[:, g0 * 128 : (g0 + g) * 128])
                    ps = ppp.tile([128, G * HF], F32)
                    for t in range(g):
                        nc.tensor.matmul(ps[:, ts(t, HF)], lhsT=xt_t[:, ts(t, 128)],
                                         rhs=w_sb[:], start=True, stop=False)
                        nc.tensor.matmul(ps[:, ts(t, HF)], lhsT=ones_sb[:], rhs=b_sb[:],
                                         start=False, stop=True)
                    ft = pp.tile([128, G, HF], BF, tag="ft")
                    nc.scalar.copy(out=ft[:, :g, :].rearrange("p g f -> p (g f)"),
                                   in_=ps[:, : g * HF])
                    for mk, t0, nt in dests:
                        a = max(g0, t0)
                        b = min(g0 + g, t0 + nt)
                        if a < b:
                            nc.sync.dma_start(out=mk(a - t0, b - t0),
                                              in_=ft[:, a - g0 : b - g0, :])
            project(xT.ap(), c.N_pad // 128, w_src_sb, b_src_sb,
                    [(lambda a, b: feat_lo[:, a:b, :], 0, c.T_lo),
                     (lambda a, b: feat_hi[:, a:b, :], c.T_lo, c.T_hi)])
            project(xT_own.ap(), c.NW, w_dst_sb, b_dst_sb,
                    [(lambda a, b: feat_doh[:, a:b, :HF], 0, c.NW)])

        # --- edge phase ---
        # feature columns are in (f, h) order: broadcasts of per-(edge,head)
        # scalars keep the packed H dim innermost -> DVE 2x everywhere.
        ep = ctx.enter_context(tc.tile_pool(name="edge", bufs=3))
        wa = ctx.enter_context(tc.tile_pool(name="workA", bufs=1))
        wb = ctx.enter_context(tc.tile_pool(name="workB", bufs=2))
        mp = ctx.enter_context(tc.tile_pool(name="msgp", bufs=3))
        op_ = ctx.enter_context(tc.tile_pool(name="outp", bufs=2))
        up = ctx.enter_context(tc.tile_pool(name="upsum", bufs=4, space="PSUM"))

        def fh(t):   # [128, B, HF] -> [128, B, F, H] view
            return t.rearrange("p b (f h) -> p b f h", h=H)

        def epilogue(w, U):
            # softmax divide + (f,h)->(h,f) permute + store, for window w
            inv = op_.tile([128, H], F32, tag="inv")
            nc.vector.tensor_scalar_max(inv[:], U[:, HF:], 1e-30)
            nc.vector.reciprocal(inv[:], inv[:])
            ot = op_.tile([128, F, H], F32, tag="ot")
            nc.vector.tensor_tensor(
                out=ot[:], in0=U[:, :HF].rearrange("p (f h) -> p f h", h=H),
                in1=bcast(inv, [list(inv.ap[0]), [0, F], [1, H]]), op=mm.mult)
            oth = op_.tile([128, H, F], F32, tag="oth")
            nc.scalar.copy(out=oth[:], in_=ot[:].rearrange("p f h -> p h f"))
            nc.sync.dma_start(out=out[ts(w, 128)], in_=oth[:])

        # PE warmers: keep the tensor engine at full p-state between scatters
        N_WARM = getattr(c, "n_warm", 0)
        if N_WARM:
            warm_a = con.tile([128, 512], F32)
            nc.vector.memset(warm_a[:], 0.0)
            wpsum = ctx.enter_context(tc.tile_pool(name="warmp", bufs=1, space="PSUM"))
            warm_o = wpsum.tile([128, 512], F32)

        def pe_warm(n):
            for _ in range(n):
                nc.tensor.matmul(warm_o[:], lhsT=warm_a[:, :128], rhs=warm_a[:],
                                 start=True, stop=True)

        pend = []
        for w in range(c.NW):
            # gather indices for this window
            il_t = ep.tile([128, c.S_lo], I16, tag="idxlo")
            nc.sync.dma_start(out=il_t[:], in_=idx_lo[w])
            ih_t = ep.tile([128, c.S_hi], I16, tag="idxhi")
            nc.sync.dma_start(out=ih_t[:], in_=idx_hi[w])
            id_t = ep.tile([128, c.S_d], I16, tag="idxd")
            nc.sync.dma_start(out=id_t[:], in_=idx_d[w])

            # gathers, chunked at GMAX idxs per call (HW ucode limit)
            def gchunk(dst_t, b0_out, tbl, it, nblk, elem):
                for j, g0 in enumerate(range(0, nblk * 128, GMAX)):
                    nb = min(GMAX, nblk * 128 - g0) // 128
                    nc.gpsimd.dma_gather(
                        dst_t[:, b0_out + g0 // 128 : b0_out + g0 // 128 + nb, :],
                        tbl.ap().rearrange("p t f -> (p t) f"),
                        it[:, j * (GMAX // 16) : j * (GMAX // 16) + nb * 8],
                        nb * 128, nb * 128, elem)
            el = ep.tile([128, c.B, HF], BF, tag="el")
            gchunk(el, 0, feat_lo, il_t, c.B_lo, HF)
            gchunk(el, c.B_lo, feat_hi, ih_t, c.B_hi, HF)
            # er + one-hot: 512B-row gather from the combined table
            erh = ep.tile([128, c.B, 2 * HF], BF, tag="erh")
            gchunk(erh, 0, feat_doh, id_t, c.B, 2 * HF)

            # DVE pipeline, all 2x bf16
            cw = c_all[:, w]                                   # [128, B, H]
            c_bc = bcast(cw, [list(cw.ap[0]), list(cw.ap[1]), [0, F], [1, H]])
            al_bc = bcast(alpha_sm, [list(alpha_sm.ap[0]), [0, c.B], [0, F], [1, H]])
            at_bc = bcast(attn_sm, [list(attn_sm.ap[0]), [0, c.B], [1, HF]])
            s = wa.tile([128, c.B, F, H], BF, tag="s")
            nc.vector.tensor_add(s[:], fh(el[:]), fh(erh[:, :, :HF]))
            nc.vector.tensor_tensor(out=s[:], in0=s[:], in1=c_bc, op=mm.mult)
            sa = wa.tile([128, c.B, F, H], BF, tag="sa")
            nc.vector.tensor_tensor(out=sa[:], in0=s[:], in1=al_bc, op=mm.mult)
            nc.vector.tensor_tensor(out=s[:], in0=s[:], in1=sa[:], op=mm.max)
            nc.vector.tensor_tensor(
                out=sa[:].rearrange("p b f h -> p b (f h)"), in0=s[:].rearrange("p b f h -> p b (f h)"),
                in1=at_bc, op=mm.mult)
            # preadd chain over f: 32 -> 16 -> 8 -> 4 -> 2 -> 1 (f32 finish)
            p16 = wa.tile([128, c.B, 16, H], BF, tag="p16")
            nc.vector.tensor_tensor(
                out=p16[:], in0=sa[:, :, :16, :], in1=sa[:, :, 16:, :], op=mm.add)
            p8 = wa.tile([128, c.B, 8, H], BF, tag="p8")
            nc.vector.tensor_tensor(
                out=p8[:], in0=p16[:, :, :8, :], in1=p16[:, :, 8:, :], op=mm.add)
            p4 = wa.tile([128, c.B, 4, H], BF, tag="p4")
            nc.vector.tensor_tensor(
                out=p4[:], in0=p8[:, :, :4, :], in1=p8[:, :, 4:, :], op=mm.add)
            p2 = wa.tile([128, c.B, 2, H], BF, tag="p2")
            nc.vector.tensor_tensor(
                out=p2[:], in0=p4[:, :, :2, :], in1=p4[:, :, 2:, :], op=mm.add)
            score = wb.tile([128, c.B, H], F32, tag="score")
            nc.vector.tensor_add(score[:], p2[:, :, 0, :], p2[:, :, 1, :])
            ex = wb.tile([128, c.B, H], F32, tag="ex")
            nc.scalar.activation(ex[:], score[:], mybir.ActivationFunctionType.Exp)
            # msgex: [el*ex | ex] (numerator cols in (f,h) order + denominator)
            msgex = mp.tile([128, c.B, HF + H], BF, tag="msgex")
            nc.scalar.copy(out=msgex[:, :, HF:], in_=ex[:])
            exd = msgex[:, :, HF:]                              # bf16 ex
            ex_bc = bcast(exd, [list(exd.ap[0]), list(exd.ap[1]), [0, F], [1, H]])
            nc.vector.tensor_tensor(
                out=fh(msgex[:, :, :HF]), in0=fh(el[:]), in1=ex_bc, op=mm.mult)

            U = up.tile([128, HF + H], F32, tag="U")
            for b in range(c.B):
                nc.tensor.matmul(U[:], lhsT=erh[:, b, HF:], rhs=msgex[:, b, :],
                                 start=(b == 0), stop=(b == c.B - 1))
            if N_WARM:
                pe_warm(N_WARM)
            pend.append((w, U))
            if len(pend) >= 3:
                epilogue(*pend.pop(0))
        for e in pend:
            epilogue(*e)

    nc.compile()
    return nc


def _run(inputs, cfg=None, trace=False):
    from concourse.bass_utils import run_bass_kernel_spmd
    x = np.asarray(inputs["x"], np.float32)
    src = np.asarray(inputs["src"]).astype(np.int64)
    dst = np.asarray(inputs["dst"]).astype(np.int64)
    if cfg is None:
        cfg = pick_cfg(src, dst, x.shape[0], 8)
    maps = host_prep(
        x, np.asarray(inputs["distance"], np.float32),
        np.asarray(inputs["W_src"], np.float32), np.asarray(inputs["b_src"], np.float32),
        np.asarray(inputs["W_dst"], np.float32), np.asarray(inputs["b_dst"], np.float32),
        np.asarray(inputs["attn"], np.float32), np.asarray(inputs["prelu_alpha"], np.float32),
        np.asarray(inputs["frequencies"], np.float32), src, dst, cfg)
    nc = build_kernel(cfg)
    res = run_bass_kernel_spmd(nc, maps, list(range(cfg.n_cores)), trace=trace)
    outs = [res.results[k]["out"] for k in range(cfg.n_cores)]
    full = np.concatenate(outs, axis=0)[: cfg.N]
    return full.reshape(cfg.N, H, F).astype(np.float32), res


def kernel(**inputs) -> np.ndarray:
    """Full-input entry point: shards across 8 NeuronCores internally."""
    out, _ = _run(inputs)
    return out


# revision 30
# speedup vs baseline: 1.3100x; 1.0262x over previous
"""GATv2 + Bessel edge-softmax kernel for TRN2, 8-core SPMD, dst-sharded.

v2 strategy (vs v1 baseline):
  - Sort edges by dst (host). Core k owns node range [k*NV, (k+1)*NV).
  - Per 128-node window: edges padded to fixed block count B, split lo/hi
    by src only for the el gather (int16 index limit).
  - Feature tables stored PARTITION-MAJOR in DRAM ([128, T, row]) so the
    projection writes stream at 4KB/descriptor; gather indices are
    host-remapped to r' = (s%128)*T + s//128.
  - er gather fused with the scatter one-hot: combined table
    [feat_dst | I] with 512B rows — one gather returns er AND the
    one-hot row used by the scatter matmul (<512B descriptors pay a 2x
    DMA penalty, so the extra 256B is free).
  - One dma_gather call per table per window (994ns SWDGE fixed cost);
    descriptor ring sized 2048 via dynamic_dma_scratch_size=32768.
  - Engine split per window: DVE add/cmul/alphamul/max/preadd/attnmul,
    Pool msgmul(broadcast ex)+reduce+gather prep, ACT c_big bcast + exp,
    PE one-hot scatter matmul into PSUM (numerator + denominator).
  - No cross-core communication (output dst-sharded, concatenated on host).
"""
import sys
sys.path.insert(0, "/opt/trn_rl_repo")
import numpy as np
import ml_dtypes
import concourse.bass as bass
import concourse.tile as tile
from concourse import bacc, mybir
from concourse.bass import ts
from contextlib import ExitStack

F32 = mybir.dt.float32
BF = mybir.dt.bfloat16
I16 = mybir.dt.int16

CUTOFF = 4.0
P_ENV = 7
ENV_A = -(P_ENV + 1) * (P_ENV + 2) / 2.0   # -36
ENV_B = float(P_ENV * (P_ENV + 2))          # 63
ENV_C = -P_ENV * (P_ENV + 1) / 2.0          # -28
H, F, HF, IN = 4, 32, 128, 128


class Cfg:
    def __init__(self, N, n_cores, NV, B_lo, B_hi, V_lo):
        self.N = N                  # real node count
        self.n_cores = n_cores
        self.NV = NV                # padded nodes per core (mult of 128)
        self.NW = NV // 128         # windows per core
        self.N_pad = NV * n_cores
        self.B_lo = B_lo            # lo-gather blocks per window
        self.B_hi = B_hi
        self.B = B_lo + B_hi
        self.V_lo = V_lo            # src table split (<= 32768, mult of 128)
        self.V_hi = self.N_pad - V_lo
        assert self.V_lo % 128 == 0 and self.V_lo <= 32768
        assert self.V_hi <= 32768
        self.T_lo = self.V_lo // 128    # partition-major col count, lo table
        self.T_hi = self.V_hi // 128
        self.T_d = self.NW + 1          # dst table cols (+1 zero pad row)
        self.S_lo = B_lo * 128 // 16    # idx cols per window
        self.S_hi = B_hi * 128 // 16
        self.S_d = self.B * 128 // 16


GMAX = 1024   # HW ucode limit: <=1024 idxs per dma_gather call


def wrap_idx(vals, nslots):
    """int16 gather index layout, chunked at GMAX idxs per call: within a
    chunk, logical i at [i%16, 64*chunk + i//16], replicated across the 8
    gpsimd groups (rows 16g+r)."""
    a = np.zeros(nslots, np.int32)
    a[: len(vals)] = vals
    assert nslots % 128 == 0
    cols = []
    for c0 in range(0, nslots, GMAX):
        chunk = a[c0 : c0 + GMAX]
        w = chunk.reshape(len(chunk) // 16, 16).T.astype(np.int16)  # [16, s]
        cols.append(np.tile(w, (8, 1)))
    return np.concatenate(cols, axis=1)                             # [128, S]


def host_prep(x, distance, W_src, b_src, W_dst, b_dst, attn, prelu_alpha,
              frequencies, src, dst, cfg: Cfg):
    """Build per-core input maps."""
    c = cfg
    order = np.argsort(dst, kind="stable")
    src_s, dst_s, dis_s = src[order], dst[order], distance[order]

    BFNP = ml_dtypes.bfloat16
    xT = np.zeros((IN, c.N_pad), BFNP)
    xT[:, : c.N] = x.T.astype(BFNP)

    # feature columns permuted to (f, h) order: new col f*H+h = old h*F+f.
    # Keeps the packed H dim innermost so per-(edge,head) broadcasts stay in
    # DVE 2x mode.
    perm = (np.arange(HF).reshape(F, H).T.reshape(-1))       # old idx for new? see below
    # old layout hf = h*F+f ; want new[f*H+h] = old[h*F+f]
    perm = np.array([[h * F + f for h in range(H)] for f in range(F)]).reshape(-1)
    smalls = dict(
        w_src_t=np.ascontiguousarray(W_src.T[:, perm].astype(BFNP)),
        w_dst_t=np.ascontiguousarray(W_dst.T[:, perm].astype(BFNP)),
        b_src_row=b_src.reshape(1, HF)[:, perm].astype(BFNP),
        b_dst_row=b_dst.reshape(1, HF)[:, perm].astype(BFNP),
        attn_row=attn.reshape(1, HF)[:, perm].astype(np.float32),
        alpha_row=prelu_alpha.reshape(1, H).astype(np.float32),
        freq_row=frequencies.reshape(1, H).astype(np.float32),
    )

    # partition-major index remap: node s -> row' = (s%128)*T + s//128
    def pm(s, T):
        return (s % 128) * T + s // 128

    core_of = dst_s // c.NV
    maps = []
    for k in range(c.n_cores):
        sel = core_of == k
        sk, dk, ik = src_s[sel], dst_s[sel] - k * c.NV, dis_s[sel]
        win = dk // 128
        idx_lo = np.zeros((c.NW, 128, c.S_lo), np.int16)
        idx_hi = np.zeros((c.NW, 128, c.S_hi), np.int16)
        idx_d = np.zeros((c.NW, 128, c.S_d), np.int16)
        dist = np.full((128, c.NW, c.B), 1.0, np.float32)

        for w in range(c.NW):
            wsel = win == w
            ws, wd, wi = sk[wsel], dk[wsel], ik[wsel]
            lo = ws < c.V_lo
            ls, li = ws[lo], wi[lo]
            hs, hi_ = ws[~lo], wi[~lo]
            ld, hd = wd[lo], wd[~lo]
            n_lo, n_hi = len(ls), len(hs)
            assert n_lo <= c.B_lo * 128, (k, w, n_lo)
            assert n_hi <= c.B_hi * 128, (k, w, n_hi)
            idx_lo[w] = wrap_idx(pm(ls, c.T_lo), c.B_lo * 128)
            idx_hi[w] = wrap_idx(pm(hs - c.V_lo, c.T_hi), c.B_hi * 128)
            # dst rows: local node d -> r' = (d%128)*T_d + d//128; the one-hot
            # part of row r' is onehot(d%128) which is what the scatter needs.
            # padding slots -> column NW (zero row) of partition 0.
            dv = np.full(c.B * 128, c.NW, np.int64)       # pad -> p=0,t=NW
            dv[:n_lo] = pm(ld, c.T_d)
            dv[c.B_lo * 128 : c.B_lo * 128 + n_hi] = pm(hd, c.T_d)
            idx_d[w] = wrap_idx(dv, c.B * 128)
            jl = np.arange(n_lo)
            dist[jl % 128, w, jl // 128] = li
            jh = np.arange(n_hi)
            dist[jh % 128, w, c.B_lo + jh // 128] = hi_

        m = dict(smalls)
        m.update(xT=xT,
                 xT_own=np.ascontiguousarray(xT[:, k * c.NV : (k + 1) * c.NV]),
                 idx_lo=idx_lo, idx_hi=idx_hi, idx_d=idx_d, dist=dist)
        maps.append(m)
    return maps


def pick_cfg(src, dst, N, n_cores=8):
    NV = ((N + n_cores * 128 - 1) // (n_cores * 128)) * 128
    N_pad = NV * n_cores
    V_lo = min(32768, N_pad // 2 // 128 * 128)
    if N_pad - V_lo > 32768:
        raise ValueError("graph too large for 2-table split")
    order = np.argsort(dst, kind="stable")
    src_s, dst_s = src[order], dst[order]
    core_of = dst_s // NV
    max_lo = max_hi = 1
    for k in range(n_cores):
        sel = core_of == k
        sk, dk = src_s[sel], dst_s[sel] - k * NV
        win = dk // 128
        for w in range(NV // 128):
            ws = sk[win == w]
            n_lo = int((ws < V_lo).sum())
            n_hi = len(ws) - n_lo
            max_lo = max(max_lo, n_lo)
            max_hi = max(max_hi, n_hi)
    B_lo = (max_lo + 127) // 128
    B_hi = (max_hi + 127) // 128
    return Cfg(N, n_cores, NV, B_lo, B_hi, V_lo)


def bcast(t, dims):
    """Manual broadcast AP from a tile/AP: dims is a list of [step, count]."""
    return bass.AP(tensor=t.tensor, offset=t.offset, ap=[list(d) for d in dims])


def build_kernel(c: Cfg):
    nc = bacc.Bacc("TRN2", target_bir_lowering=False, debug=False)
    dp = nc.declare_dram_parameter
    xT = dp("xT", [IN, c.N_pad], BF, isOutput=False)
    xT_own = dp("xT_own", [IN, c.NV], BF, isOutput=False)
    w_src_t = dp("w_src_t", [IN, HF], BF, isOutput=False)
    w_dst_t = dp("w_dst_t", [IN, HF], BF, isOutput=False)
    b_src_row = dp("b_src_row", [1, HF], BF, isOutput=False)
    b_dst_row = dp("b_dst_row", [1, HF], BF, isOutput=False)
    attn_row = dp("attn_row", [1, HF], F32, isOutput=False)
    alpha_row = dp("alpha_row", [1, H], F32, isOutput=False)
    freq_row = dp("freq_row", [1, H], F32, isOutput=False)
    idx_lo = dp("idx_lo", [c.NW, 128, c.S_lo], I16, isOutput=False)
    idx_hi = dp("idx_hi", [c.NW, 128, c.S_hi], I16, isOutput=False)
    idx_d = dp("idx_d", [c.NW, 128, c.S_d], I16, isOutput=False)
    dist = dp("dist", [128, c.NW, c.B], F32, isOutput=False)
    out = dp("out", [c.NV, HF], F32, isOutput=True)

    # partition-major feature tables
    feat_lo = nc.dram_tensor("feat_lo", [128, c.T_lo, HF], BF)
    feat_hi = nc.dram_tensor("feat_hi", [128, c.T_hi, HF], BF)
    # combined dst table: [er row | one-hot row]; col NW is the zero pad row
    feat_doh = nc.dram_tensor("feat_doh", [128, c.T_d, 2 * HF], BF)

    NWB = c.NW * c.B
    mm = mybir.AluOpType

    with tile.TileContext(nc, pool_alloc_mode="queue") as tc, ExitStack() as ctx:
        con = ctx.enter_context(tc.tile_pool(name="con", bufs=1))
        # --- constants ---
        attn_rep = con.tile([128, HF], F32)
        nc.sync.dma_start(out=attn_rep[:], in_=bcast(attn_row.ap(), [[0, 128], [1, HF]]))
        alpha_rep = con.tile([128, H], F32)
        nc.sync.dma_start(out=alpha_rep[:], in_=bcast(alpha_row.ap(), [[0, 128], [1, H]]))
        freq_rep = con.tile([128, H], F32)
        nc.sync.dma_start(out=freq_rep[:], in_=bcast(freq_row.ap(), [[0, 128], [1, H]]))
        # bf16 copies; broadcast over the block dim via stride-0 APs (the
        # packed innermost dim keeps DVE 2x)
        attn_sm = con.tile([128, HF], BF)       # (f,h) order
        nc.vector.tensor_copy(out=attn_sm[:], in_=attn_rep[:])
        alpha_sm = con.tile([128, H], BF)
        nc.vector.tensor_copy(out=alpha_sm[:], in_=alpha_rep[:])
        w_src_sb = con.tile([IN, HF], BF)
        nc.sync.dma_start(out=w_src_sb[:], in_=w_src_t[:])
        w_dst_sb = con.tile([IN, HF], BF)
        nc.sync.dma_start(out=w_dst_sb[:], in_=w_dst_t[:])
        ones_sb = con.tile([1, 128], BF)
        nc.vector.memset(ones_sb[:], 1.0)
        b_src_sb = con.tile([1, HF], BF)
        nc.sync.dma_start(out=b_src_sb[:], in_=b_src_row[:])
        b_dst_sb = con.tile([1, HF], BF)
        nc.sync.dma_start(out=b_dst_sb[:], in_=b_dst_row[:])
        # one-hot identity block for the dst table: I[p, j] = (p == j)
        iota_i = con.tile([128, 128], mybir.dt.int32)
        nc.gpsimd.iota(iota_i[:], pattern=[[1, 128]], base=0, channel_multiplier=0)
        iota_f = con.tile([128, 128], BF)
        nc.vector.tensor_copy(out=iota_f[:], in_=iota_i[:])
        pid_i = con.tile([128, 1], mybir.dt.int32)
        nc.gpsimd.iota(pid_i[:], pattern=[[0, 1]], base=0, channel_multiplier=1)
        pid_f = con.tile([128, 1], BF)
        nc.vector.tensor_copy(out=pid_f[:], in_=pid_i[:])
        eye_sb = con.tile([128, 128], BF)
        nc.vector.tensor_tensor(
            out=eye_sb[:], in0=iota_f[:],
            in1=bcast(pid_f, [list(pid_f.ap[0]), [0, 128]]), op=mm.is_equal)
        zero_row = con.tile([128, 2 * HF], BF)
        nc.vector.memset(zero_row[:], 0.0)
        c_all = con.tile([128, c.NW, c.B, H], BF)

        # write the one-hot halves + zero pad column of feat_doh once
        # (materialized copy: stride-0-source DMA descriptors are not safe)
        with tc.tile_pool(name="eyep", bufs=1) as eyp:
            eye_rep = eyp.tile([128, c.NW, 128], BF)
            nc.vector.tensor_copy(
                out=eye_rep[:],
                in_=bcast(eye_sb, [list(eye_sb.ap[0]), [0, c.NW], [1, 128]]))
            nc.sync.dma_start(out=feat_doh[:, : c.NW, HF:], in_=eye_rep[:])
            nc.sync.dma_start(out=feat_doh[:, c.NW, :], in_=zero_row[:])

        # --- coefficient pre-pass (Sin table) ---
        with tc.tile_pool(name="coef", bufs=1) as cp:
            d_t = cp.tile([128, NWB], F32)
            nc.sync.dma_start(out=d_t[:], in_=dist.ap().rearrange("p w b -> p (w b)"))
            nc.vector.tensor_scalar_mul(d_t[:], d_t[:], 1.0 / CUTOFF)
            d2 = cp.tile([128, NWB], F32)
            nc.vector.tensor_mul(d2[:], d_t[:], d_t[:])
            d3 = cp.tile([128, NWB], F32)
            nc.vector.tensor_mul(d3[:], d2[:], d_t[:])
            d7 = cp.tile([128, NWB], F32)
            nc.vector.tensor_mul(d7[:], d3[:], d3[:])          # d6
            nc.vector.tensor_mul(d7[:], d7[:], d_t[:])          # d7
            inner = cp.tile([128, NWB], F32)
            nc.vector.tensor_scalar(inner[:], d_t[:], ENV_B, ENV_A, op0=mm.mult, op1=mm.add)
            nc.vector.scalar_tensor_tensor(inner[:], d2[:], ENV_C, inner[:], op0=mm.mult, op1=mm.add)
            env = cp.tile([128, NWB], F32)
            nc.vector.tensor_mul(env[:], d7[:], inner[:])
            nc.vector.tensor_add(env[:], env[:], d_t[:])
            targ = cp.tile([128, NWB, H], F32)
            nc.vector.tensor_tensor(
                out=targ[:], in0=d_t[:].to_broadcast([128, NWB, H]),
                in1=bcast(freq_rep, [list(freq_rep.ap[0]), [0, NWB], [1, H]]),
                op=mm.mult)
            # Sin on ACT only valid in [-pi, pi]; args reach 4*pi -> wrap twice
            nc.vector.add_range_wrap(targ[:], targ[:], 0.0, float(np.pi), float(2 * np.pi))
            nc.vector.add_range_wrap(targ[:], targ[:], 0.0, float(np.pi), float(2 * np.pi))
            sinv = cp.tile([128, NWB, H], F32)
            nc.scalar.activation(sinv[:], targ[:], mybir.ActivationFunctionType.Sin)
            nc.vector.tensor_tensor(
                out=c_all[:].rearrange("p w b h -> p (w b) h"), in0=sinv[:],
                in1=env[:].to_broadcast([128, NWB, H]),
                op=mm.mult)

        # --- projections (partition-major tables) ---
        with tc.tile_pool(name="proj", bufs=3) as pp, \
             tc.tile_pool(name="projp", bufs=2, space="PSUM") as ppp:
            def project(src_ap, n_tiles, w_sb, b_sb, dests):
                # dests: list of (dram_slice_fn, t0, nt); dram_slice_fn(a, b)
                # yields the out AP for table cols [a-t0, b-t0)
                G = 16
                for g0 in range(0, n_tiles, G):
                    g = min(G, n_tiles - g0)
                    xt_t = pp.tile([128, G * 128], BF, tag="xt")
                    nc.sync.dma_start(
                        out=xt_t[:, : g * 128],
                        in_=src_ap[:, g0 * 128 : (g0 + g) * 128])
                    ps = ppp.tile([128, G * HF], F32)
                    for t in range(g):
                        nc.tensor.matmul(ps[:, ts(t, HF)], lhsT=xt_t[:, ts(t, 128)],
                                         rhs=w_sb[:], start=True, stop=False)
                        nc.tensor.matmul(ps[:, ts(t, HF)], lhsT=ones_sb[:], rhs=b_sb[:],
                                         start=False, stop=True)
                    ft = pp.tile([128, G, HF], BF, tag="ft")
                    nc.scalar.copy(out=ft[:, :g, :].rearrange("p g f -> p (g f)"),
                                   in_=ps[:, : g * HF])
                    for mk, t0, nt in dests:
                        a = max(g0, t0)
                        b = min(g0 + g, t0 + nt)
                        if a < b:
                            nc.sync.dma_start(out=mk(a - t0, b - t0),
                                              in_=ft[:, a - g0 : b - g0, :])
            # dst table first: erh gathers for early windows can start while
            # the (much larger) src projection is still streaming
            project(xT_own.ap(), c.NW, w_dst_sb, b_dst_sb,
                    [(lambda a, b: feat_doh[:, a:b, :HF], 0, c.NW)])
            project(xT.ap(), c.N_pad // 128, w_src_sb, b_src_sb,
                    [(lambda a, b: feat_lo[:, a:b, :], 0, c.T_lo),
                     (lambda a, b: feat_hi[:, a:b, :], c.T_lo, c.T_hi)])

        # --- edge phase ---
        # feature columns are in (f, h) order: broadcasts of per-(edge,head)
        # scalars keep the packed H dim innermost -> DVE 2x everywhere.
        ep = ctx.enter_context(tc.tile_pool(name="edge", bufs=3))
        wa = ctx.enter_context(tc.tile_pool(name="workA", bufs=1))
        wb = ctx.enter_context(tc.tile_pool(name="workB", bufs=2))
        mp = ctx.enter_context(tc.tile_pool(name="msgp", bufs=3))
        op_ = ctx.enter_context(tc.tile_pool(name="outp", bufs=2))
        up = ctx.enter_context(tc.tile_pool(name="upsum", bufs=4, space="PSUM"))

        def fh(t):   # [128, B, HF] -> [128, B, F, H] view
            return t.rearrange("p b (f h) -> p b f h", h=H)

        def epilogue(w, U):
            # softmax divide + (f,h)->(h,f) permute + store, for window w
            inv = op_.tile([128, H], F32, tag="inv")
            nc.vector.tensor_scalar_max(inv[:], U[:, HF:], 1e-30)
            nc.vector.reciprocal(inv[:], inv[:])
            ot = op_.tile([128, F, H], F32, tag="ot")
            nc.vector.tensor_tensor(
                out=ot[:], in0=U[:, :HF].rearrange("p (f h) -> p f h", h=H),
                in1=bcast(inv, [list(inv.ap[0]), [0, F], [1, H]]), op=mm.mult)
            oth = op_.tile([128, H, F], F32, tag="oth")
            nc.scalar.copy(out=oth[:], in_=ot[:].rearrange("p f h -> p h f"))
            nc.sync.dma_start(out=out[ts(w, 128)], in_=oth[:])

        # PE warmers: keep the tensor engine at full p-state between scatters
        N_WARM = getattr(c, "n_warm", 0)
        if N_WARM:
            warm_a = con.tile([128, 512], F32)
            nc.vector.memset(warm_a[:], 0.0)
            wpsum = ctx.enter_context(tc.tile_pool(name="warmp", bufs=1, space="PSUM"))
            warm_o = wpsum.tile([128, 512], F32)

        def pe_warm(n):
            for _ in range(n):
                nc.tensor.matmul(warm_o[:], lhsT=warm_a[:, :128], rhs=warm_a[:],
                                 start=True, stop=True)

        pend = []
        for w in range(c.NW):
            # gather indices for this window
            il_t = ep.tile([128, c.S_lo], I16, tag="idxlo")
            nc.sync.dma_start(out=il_t[:], in_=idx_lo[w])
            ih_t = ep.tile([128, c.S_hi], I16, tag="idxhi")
            nc.sync.dma_start(out=ih_t[:], in_=idx_hi[w])
            id_t = ep.tile([128, c.S_d], I16, tag="idxd")
            nc.sync.dma_start(out=id_t[:], in_=idx_d[w])

            # gathers, chunked at GMAX idxs per call (HW ucode limit)
            def gchunk(dst_t, b0_out, tbl, it, nblk, elem):
                for j, g0 in enumerate(range(0, nblk * 128, GMAX)):
                    nb = min(GMAX, nblk * 128 - g0) // 128
                    nc.gpsimd.dma_gather(
                        dst_t[:, b0_out + g0 // 128 : b0_out + g0 // 128 + nb, :],
                        tbl.ap().rearrange("p t f -> (p t) f"),
                        it[:, j * (GMAX // 16) : j * (GMAX // 16) + nb * 8],
                        nb * 128, nb * 128, elem)
            el = ep.tile([128, c.B, HF], BF, tag="el")
            gchunk(el, 0, feat_lo, il_t, c.B_lo, HF)
            gchunk(el, c.B_lo, feat_hi, ih_t, c.B_hi, HF)
            # er + one-hot: 512B-row gather from the combined table
            erh = ep.tile([128, c.B, 2 * HF], BF, tag="erh")
            gchunk(erh, 0, feat_doh, id_t, c.B, 2 * HF)

            # DVE pipeline, all 2x bf16
            cw = c_all[:, w]                                   # [128, B, H]
            c_bc = bcast(cw, [list(cw.ap[0]), list(cw.ap[1]), [0, F], [1, H]])
            al_bc = bcast(alpha_sm, [list(alpha_sm.ap[0]), [0, c.B], [0, F], [1, H]])
            at_bc = bcast(attn_sm, [list(attn_sm.ap[0]), [0, c.B], [1, HF]])
            s = wa.tile([128, c.B, F, H], BF, tag="s")
            nc.vector.tensor_add(s[:], fh(el[:]), fh(erh[:, :, :HF]))
            nc.vector.tensor_tensor(out=s[:], in0=s[:], in1=c_bc, op=mm.mult)
            sa = wa.tile([128, c.B, F, H], BF, tag="sa")
            nc.vector.tensor_tensor(out=sa[:], in0=s[:], in1=al_bc, op=mm.mult)
            nc.vector.tensor_tensor(out=s[:], in0=s[:], in1=sa[:], op=mm.max)
            nc.vector.tensor_tensor(
                out=sa[:].rearrange("p b f h -> p b (f h)"), in0=s[:].rearrange("p b f h -> p b (f h)"),
                in1=at_bc, op=mm.mult)
            # preadd chain over f: 32 -> 16 -> 8 -> 4 -> 2 -> 1 (f32 finish)
            p16 = wa.tile([128, c.B, 16, H], BF, tag="p16")
            nc.vector.tensor_tensor(
                out=p16[:], in0=sa[:, :, :16, :], in1=sa[:, :, 16:, :], op=mm.add)
            p8 = wa.tile([128, c.B, 8, H], BF, tag="p8")
            nc.vector.tensor_tensor(
                out=p8[:], in0=p16[:, :, :8, :], in1=p16[:, :, 8:, :], op=mm.add)
            p4 = wa.tile([128, c.B, 4, H], BF, tag="p4")
            nc.vector.tensor_tensor(
                out=p4[:], in0=p8[:, :, :4, :], in1=p8[:, :, 4:, :], op=mm.add)
            p2 = wa.tile([128, c.B, 2, H], BF, tag="p2")
            nc.vector.tensor_tensor(
                out=p2[:], in0=p4[:, :, :2, :], in1=p4[:, :, 2:, :], op=mm.add)
            score = wb.tile([128, c.B, H], F32, tag="score")
            nc.vector.tensor_add(score[:], p2[:, :, 0, :], p2[:, :, 1, :])
            ex = wb.tile([128, c.B, H], F32, tag="ex")
            nc.scalar.activation(ex[:], score[:], mybir.ActivationFunctionType.Exp)
            # msgex: [el*ex | ex] (numerator cols in (f,h) order + denominator)
            msgex = mp.tile([128, c.B, HF + H], BF, tag="msgex")
            nc.scalar.copy(out=msgex[:, :, HF:], in_=ex[:])
            exd = msgex[:, :, HF:]                              # bf16 ex
            ex_bc = bcast(exd, [list(exd.ap[0]), list(exd.ap[1]), [0, F], [1, H]])
            nc.vector.tensor_tensor(
                out=fh(msgex[:, :, :HF]), in0=fh(el[:]), in1=ex_bc, op=mm.mult)

            U = up.tile([128, HF + H], F32, tag="U")
            for b in range(c.B):
                nc.tensor.matmul(U[:], lhsT=erh[:, b, HF:], rhs=msgex[:, b, :],
                                 start=(b == 0), stop=(b == c.B - 1))
            if N_WARM:
                pe_warm(N_WARM)
            pend.append((w, U))
            if len(pend) >= 3:
                epilogue(*pend.pop(0))
        for e in pend:
            epilogue(*e)

    nc.compile()
    return nc


def _run(inputs, cfg=None, trace=False):
    from concourse.bass_utils import run_bass_kernel_spmd
    x = np.asarray(inputs["x"], np.float32)
    src = np.asarray(inputs["src"]).astype(np.int64)
    dst = np.asarray(inputs["dst"]).astype(np.int64)
    if cfg is None:
        cfg = pick_cfg(src, dst, x.shape[0], 8)
    maps = host_prep(
        x, np.asarray(inputs["distance"], np.float32),
        np.asarray(inputs["W_src"], np.float32), np.asarray(inputs["b_src"], np.float32),
        np.asarray(inputs["W_dst"], np.float32), np.asarray(inputs["b_dst"], np.float32),
        np.asarray(inputs["attn"], np.float32), np.asarray(inputs["prelu_alpha"], np.float32),
        np.asarray(inputs["frequencies"], np.float32), src, dst, cfg)
    nc = build_kernel(cfg)
    res = run_bass_kernel_spmd(nc, maps, list(range(cfg.n_cores)), trace=trace)
    outs = [res.results[k]["out"] for k in range(cfg.n_cores)]
    full = np.concatenate(outs, axis=0)[: cfg.N]
    return full.reshape(cfg.N, H, F).astype(np.float32), res


def kernel(**inputs) -> np.ndarray:
    """Full-input entry point: shards across 8 NeuronCores internally."""
    out, _ = _run(inputs)
    return out
